# revision 1
# baseline (speedup 1.0000x reference)
"""Trainium2 Bass kernel for CustomTransformerBlock (S=2048, B=4, H=2048, NH=16).

Sharding: sequence-zigzag token parallelism across 8 cores.
  - seq blocks of 128: core c owns blocks {c, 15-c}  (causal-balanced)
  - LN1/QKV/proj/LN2/MLP are token-parallel (no communication)
  - K,V all-gathered (bf16) once; attention computed on own query tokens
  - all matmuls bf16 with fp32 PSUM accumulation; residual spine fp32
Layout: activations feature-major [H, tokens]; scores computed transposed
  S^T=[k,q] so softmax denominator comes from an N=1 matmul and PV needs no
  P transpose. Uniform SPMD program: causal variation is data-driven via a
  per-core exp-bias table (padded tiles get bias -1e5 -> exp==0).
"""

import sys
import types

for _p in ("/opt/trn_rl_repo", "/root/.axon_site", "/root/.axon_site/_ro/trn_rl_repo"):
    if _p not in sys.path:
        sys.path.append(_p)

import numpy as np
import ml_dtypes
from contextlib import ExitStack

import concourse.bass as bass
import concourse.tile as tile
from concourse import bacc, mybir
from concourse import bass_utils

BF = ml_dtypes.bfloat16
DTB = mybir.dt.bfloat16
DTF = mybir.dt.float32
AF = mybir.ActivationFunctionType

NCORE = 8
EXP_C = 12.0
MASKV = -1.0e5

FULL = dict(S=2048, B=4, H=2048, NH=16, FF=8192)

_LAST = {}


def _shim_ntff_hook():
    if "antenv.axon_hooks" in sys.modules:
        return
    holder = {"h": None}
    mod = types.ModuleType("antenv.axon_hooks")
    mod.set_axon_ntff_profile_hook = lambda h: holder.__setitem__("h", h)
    mod.get_axon_ntff_profile_hook = lambda: holder["h"]
    sys.modules["antenv.axon_hooks"] = mod
    try:
        import antenv

        antenv.axon_hooks = mod
        from trn_agent_boot.trn_boot import _ntff_profile_via_ctypes

        mod.set_axon_ntff_profile_hook(
            _ntff_profile_via_ctypes("/opt/axon/libaxon_pjrt.so")
        )
    except Exception:
        pass


def build(cfg):
    """Build + compile the SPMD Bass program. Returns nc."""
    S, B, H, NH, FF = cfg["S"], cfg["B"], cfg["H"], cfg["NH"], cfg["FF"]
    HD = 128
    assert H == NH * HD or True  # NH*HD may be < H in small cfg? no: keep ==
    KT = H // 128  # feature k-tiles
    OT = H // 128  # output feature tiles (H)
    FT = FF // 128  # fc1 out tiles
    TPC = 2 * B * 128  # tokens per core
    NCOL = TPC // 512 if TPC >= 512 else 1  # 512-token col blocks
    CW = min(512, TPC)  # col width
    NBLK = 2 * NCORE  # 16 seq blocks
    PAD0, PAD1 = NCORE - 1, NBLK - 1  # off-diag loop bounds 7, 15
    invH = 1.0 / H

    nc = bacc.Bacc("TRN2", target_bir_lowering=False, debug=False, num_devices=NCORE)

    def din(name, shape, dt=DTB):
        return nc.dram_tensor(name, list(shape), dt, kind="ExternalInput").ap()

    x_bf = din("x_bf", (H, TPC))
    x_f32 = din("x_f32", (H, TPC), DTF)
    wqT = din("wqT", (H, H))
    wkT = din("wkT", (H, H))
    wvT = din("wvT", (H, H))
    projT = din("projT", (H, H))
    fc1T = din("fc1T", (H, FF))
    fc2T = din("fc2T", (FF, H))
    qb_d = din("qb", (128, OT), DTF)
    kb_d = din("kb", (128, OT), DTF)
    vb_d = din("vb", (1, H))
    pb_d = din("pb", (128, OT), DTF)
    f1b_d = din("f1b", (128, FT), DTF)
    f2b_d = din("f2b", (128, OT), DTF)
    tri_d = din("tri", (128, 128), DTF)
    bpad_d = din("bpad", (128, PAD0 + PAD1 + 1), DTF)
    ones_r_d = din("ones_r", (1, 512))
    ones_f_d = din("ones_f", (1, 128), DTF)
    ones_c_d = din("ones_c", (128, 1))

    y_out = nc.dram_tensor("y", [H, TPC], DTF, kind="ExternalOutput").ap()

    with tile.TileContext(nc) as tc, ExitStack() as ctx:
        dram = ctx.enter_context(tc.tile_pool(name="dram", bufs=1, space="DRAM"))
        bounce_k = dram.tile([H, TPC], DTB)
        bounce_v = dram.tile([TPC, H], DTB)
        kag = dram.tile([NCORE * H, TPC], DTB)
        vag = dram.tile([NCORE * TPC, H], DTB)
        r2_dram = dram.tile([H, TPC], DTF)

        cpool = ctx.enter_context(tc.tile_pool(name="const", bufs=1))
        wpool = ctx.enter_context(tc.tile_pool(name="w", bufs=2))

        def open_pool(name, side="left"):
            cm = tc.tile_pool(name=name, bufs=1, side=side)
            return cm, cm.__enter__()
        stage = ctx.enter_context(tc.tile_pool(name="stage", bufs=3))
        scr = ctx.enter_context(tc.tile_pool(name="scr", bufs=2))
        stat = ctx.enter_context(tc.tile_pool(name="stat", bufs=1))

        # ---- constants to SBUF ----
        ones_r = cpool.tile([1, 512], DTB)
        nc.sync.dma_start(ones_r[:], ones_r_d[:, :512])
        ones_f = cpool.tile([1, 128], DTF)
        nc.sync.dma_start(ones_f[:], ones_f_d[:])
        ones_c = cpool.tile([128, 1], DTB)
        nc.sync.dma_start(ones_c[:], ones_c_d[:])
        tri = cpool.tile([128, 128], DTF)
        nc.sync.dma_start(tri[:], tri_d[:])
        bpad = cpool.tile([128, PAD0 + PAD1 + 1], DTF)
        nc.sync.dma_start(bpad[:], bpad_d[:])
        qb = cpool.tile([128, OT], DTF)
        nc.sync.dma_start(qb[:], qb_d[:])
        kb = cpool.tile([128, OT], DTF)
        nc.sync.dma_start(kb[:], kb_d[:])
        vb = cpool.tile([1, H], DTB)
        nc.sync.dma_start(vb[:], vb_d[:])
        pb = cpool.tile([128, OT], DTF)
        nc.sync.dma_start(pb[:], pb_d[:])
        f1b = cpool.tile([128, FT], DTF)
        nc.sync.dma_start(f1b[:], f1b_d[:])
        f2b = cpool.tile([128, OT], DTF)
        nc.sync.dma_start(f2b[:], f2b_d[:])

        def load_slab(ws, dr, col, n=128):
            """weight slab: dram [R, C] cols [col,col+n) -> ws [128, R/128 * n]"""
            nc.sync.dma_start(
                ws[:].rearrange("p (k f) -> p k f", f=n),
                dr[:, col : col + n].rearrange("(k p) f -> p k f", p=128),
            )

        def ln_fm(src_sb, out_sb, psStat, psB):
            """feature-major layernorm: src [128, KT*TPC] bf16 -> out bf16"""
            for c in range(NCOL):
                cs = c * CW
                ps_s = psStat.tile([1, CW], DTF, tag="ps_s")
                ps_q = psStat.tile([1, CW], DTF, tag="ps_q")
                for k in range(KT):
                    sq = scr.tile([128, CW], DTB, tag="lnsq")
                    nc.scalar.square(
                        sq[:], src_sb[:, k * TPC + cs : k * TPC + cs + CW]
                    )
                    nc.tensor.matmul(
                        ps_s[:],
                        lhsT=ones_c[:],
                        rhs=src_sb[:, k * TPC + cs : k * TPC + cs + CW],
                        start=(k == 0),
                        stop=(k == KT - 1),
                        skip_group_check=True,
                    )
                    nc.tensor.matmul(
                        ps_q[:],
                        lhsT=ones_c[:],
                        rhs=sq[:],
                        start=(k == 0),
                        stop=(k == KT - 1),
                        skip_group_check=True,
                    )
                mu = stat.tile([1, CW], DTF, tag="mu")
                nc.vector.tensor_scalar_mul(mu[:], ps_s[:], invH)
                msq = stat.tile([1, CW], DTF, tag="msq")
                nc.vector.tensor_scalar_mul(msq[:], ps_q[:], invH)
                mu2 = stat.tile([1, CW], DTF, tag="mu2")
                nc.vector.tensor_mul(mu2[:], mu[:], mu[:])
                var = stat.tile([1, CW], DTF, tag="var")
                nc.vector.tensor_sub(var[:], msq[:], mu2[:])
                nc.vector.tensor_scalar_add(var[:], var[:], 1e-5)
                sd = stat.tile([1, CW], DTF, tag="sd")
                nc.scalar.activation(sd[:], var[:], AF.Sqrt)
                rstd = stat.tile([1, CW], DTF, tag="rstd")
                nc.vector.reciprocal(rstd[:], sd[:])
                mrs = stat.tile([1, CW], DTF, tag="mrs")
                nc.vector.tensor_mul(mrs[:], mu[:], rstd[:])
                ps_r = psB.tile([128, CW], DTF, tag="ps_r")
                ps_m = psB.tile([128, CW], DTF, tag="ps_m")
                nc.tensor.matmul(ps_r[:], lhsT=ones_f[:], rhs=rstd[:])
                nc.tensor.matmul(ps_m[:], lhsT=ones_f[:], rhs=mrs[:])
                for k in range(KT):
                    t1 = scr.tile([128, CW], DTF, tag="lnt1")
                    nc.vector.tensor_mul(
                        t1[:], src_sb[:, k * TPC + cs : k * TPC + cs + CW], ps_r[:]
                    )
                    nc.vector.tensor_sub(
                        out_sb[:, k * TPC + cs : k * TPC + cs + CW], t1[:], ps_m[:]
                    )

        # ================= phase 1: LN1 =================
        px_cm, px = open_pool("px", "left")
        ph1_cm, ph1 = open_pool("ph1", "right")
        x_sb = px.tile([128, KT * TPC], DTB)
        for k in range(KT):
            nc.sync.dma_start(
                x_sb[:, k * TPC : (k + 1) * TPC], x_bf[k * 128 : (k + 1) * 128, :]
            )
        h1 = ph1.tile([128, KT * TPC], DTB)
        with tc.tile_pool(name="lnS1", bufs=1, space="PSUM") as pS, tc.tile_pool(
            name="lnB1", bufs=1, space="PSUM"
        ) as pB:
            ln_fm(x_sb, h1, pS, pB)
        px_cm.__exit__(None, None, None)

        # ================= phase 2: K, V, Q =================
        def fm_proj(wT_d, bias_sb, ps512, dst_sb=None, dst_dram=None, nt=None):
            """out[o] = wT.T @ h1 + b  (feature-major out), o-tiles of 128"""
            for o in range(nt):
                ws = wpool.tile([128, KT * 128], DTB, tag="wslab")
                load_slab(ws, wT_d, o * 128)
                for c in range(NCOL):
                    cs = c * CW
                    ps = ps512.tile([128, CW], DTF, tag="ps")  # pool arg
                    for k in range(KT):
                        nc.tensor.matmul(
                            ps[:],
                            lhsT=ws[:, k * 128 : (k + 1) * 128],
                            rhs=h1[:, k * TPC + cs : k * TPC + cs + CW],
                            start=(k == 0),
                            stop=(k == KT - 1),
                        )
                    if dst_sb is not None:
                        nc.scalar.activation(
                            dst_sb[:, o * TPC + cs : o * TPC + cs + CW],
                            ps[:], AF.Identity, bias=bias_sb[:, o : o + 1],
                        )
                    else:
                        st = stage.tile([128, CW], DTB, tag="fmstage")
                        nc.scalar.activation(
                            st[:], ps[:], AF.Identity, bias=bias_sb[:, o : o + 1]
                        )
                        nc.sync.dma_start(
                            dst_dram[o * 128 : (o + 1) * 128, cs : cs + CW], st[:]
                        )

        qkv_ps_cm = tc.tile_pool(name="qkvps", bufs=4, space="PSUM")
        qkv_ps = qkv_ps_cm.__enter__()
        fm_proj(wkT, kb, qkv_ps, dst_dram=bounce_k, nt=OT)
        # V: token-major out
        for vc in range(H // 512):
            ws = wpool.tile([128, max(KT * 512, FT * 128)], DTB, tag="bigslab", bufs=2, name="wv")[:, : KT * 512]
            load_slab(ws, wvT, vc * 512, 512)
            for m in range(TPC // 128):
                ps = qkv_ps.tile([128, 512], DTF, tag="ps")
                for k in range(KT):
                    nc.tensor.matmul(
                        ps[:],
                        lhsT=h1[:, k * TPC + m * 128 : k * TPC + (m + 1) * 128],
                        rhs=ws[:, k * 512 : (k + 1) * 512],
                        start=(k == 0),
                        stop=False,
                    )
                nc.tensor.matmul(
                    ps[:],
                    lhsT=ones_r[:, :128],
                    rhs=vb[:, vc * 512 : (vc + 1) * 512],
                    start=False,
                    stop=True,
                )
                st = stage.tile([128, 512], DTB, tag="vstage")
                nc.scalar.copy(st[:], ps[:])
                nc.sync.dma_start(
                    bounce_v[m * 128 : (m + 1) * 128, vc * 512 : (vc + 1) * 512], st[:]
                )

        # all-gather K and V while Q computes
        nc.gpsimd.collective_compute(
            "AllGather",
            mybir.AluOpType.bypass,
            replica_groups=[list(range(NCORE))],
            ins=[bounce_k.opt()],
            outs=[kag.opt()],
        )
        nc.gpsimd.collective_compute(
            "AllGather",
            mybir.AluOpType.bypass,
            replica_groups=[list(range(NCORE))],
            ins=[bounce_v.opt()],
            outs=[vag.opt()],
        )

        pq_cm, pq = open_pool("pq", "left")
        q_all = pq.tile([128, OT * TPC], DTB)
        fm_proj(wqT, qb, qkv_ps, dst_sb=q_all, nt=OT)
        qkv_ps_cm.__exit__(None, None, None)
        ph1_cm.__exit__(None, None, None)

        # ================= phase 3: attention =================
        HG = 2  # heads per slab group
        NHG = NH // HG
        pat_cm, pat = open_pool("pat", "right")
        aslab_cm = tc.tile_pool(name="aslab", bufs=2, side="right")
        aslab = aslab_cm.__enter__()
        attn = pat.tile([128, OT * TPC], DTB)
        att_ps_cm = tc.tile_pool(name="attps", bufs=1, space="PSUM")
        att_ps = att_ps_cm.__enter__()
        npads = (PAD0 + 1, PAD1 + 1)

        def kv_src(slot, b):
            """-> (k_src_ap, v_src_ap) for slab slot: 0..PAD1-1 off-diag kb=slot,
            PAD1 -> diag chunk0, PAD1+1 -> diag chunk1 (local bounce)."""
            if slot < PAD1:
                kbk = slot
                j, ck = (kbk, 0) if kbk < NCORE else (NBLK - 1 - kbk, 1)
                kc = ck * (TPC // 2) + b * 128
                return (
                    kag[j * H : (j + 1) * H, kc : kc + 128],
                    vag[j * TPC + kc : j * TPC + kc + 128, :],
                )
            ck = slot - PAD1
            kc = ck * (TPC // 2) + b * 128
            return (
                bounce_k[:, kc : kc + 128],
                bounce_v[kc : kc + 128, :],
            )

        for b in range(B):
            for hg in range(NHG):
                ks = aslab.tile([128, (PAD1 + 2) * HG * 128], DTB, tag="kslab")
                vs = aslab.tile([128, (PAD1 + 2) * HG * 128], DTB, tag="vslab")
                W = HG * 128
                for slot in range(PAD1 + 2):
                    ksrc, vsrc = kv_src(slot, b)
                    nc.sync.dma_start(
                        ks[:, slot * W : (slot + 1) * W].rearrange(
                            "p (h t) -> p h t", t=128
                        ),
                        ksrc[hg * W : (hg + 1) * W, :].rearrange(
                            "(h p) t -> p h t", p=128
                        ),
                    )
                    nc.sync.dma_start(
                        vs[:, slot * W : (slot + 1) * W],
                        vsrc[:, hg * W : (hg + 1) * W],
                    )
                for hi in range(HG):
                    h = hg * HG + hi
                    for chunk in range(2):
                        niter = npads[chunk]
                        qs = q_all[
                            :,
                            h * TPC
                            + chunk * (TPC // 2)
                            + b * 128 : h * TPC
                            + chunk * (TPC // 2)
                            + b * 128
                            + 128,
                        ]
                        ps_o = att_ps.tile([128, 128], DTF, tag="ps_o", bufs=2)
                        ps_d = att_ps.tile([128, 1], DTF, tag="ps_d", bufs=2)
                        for i in range(niter):
                            diag = i == niter - 1
                            slot = (PAD1 + chunk) if diag else i
                            st = att_ps.tile([128, 128], DTF, tag="ps_st", bufs=3)
                            nc.tensor.matmul(
                                st[:],
                                lhsT=ks[
                                    :,
                                    slot * W + hi * 128 : slot * W + hi * 128 + 128,
                                ],
                                rhs=qs,
                                start=True,
                                stop=True,
                                skip_group_check=True,
                            )
                            pt = scr.tile([128, 128], DTB, tag="pt")
                            if diag:
                                nc.vector.tensor_add(st[:], st[:], tri[:])
                                dcol = PAD0 + PAD1
                                nc.scalar.activation(
                                    pt[:], st[:], AF.Exp,
                                    bias=bpad[:, dcol : dcol + 1],
                                )
                            else:
                                col = i if chunk == 0 else PAD0 + i
                                nc.scalar.activation(
                                    pt[:], st[:], AF.Exp, bias=bpad[:, col : col + 1]
                                )
                            nc.tensor.matmul(
                                ps_d[:],
                                lhsT=pt[:],
                                rhs=ones_c[:],
                                start=(i == 0),
                                stop=(i == niter - 1),
                                skip_group_check=True,
                            )
                            nc.tensor.matmul(
                                ps_o[:],
                                lhsT=pt[:],
                                rhs=vs[
                                    :,
                                    slot * W + hi * 128 : slot * W + hi * 128 + 128,
                                ],
                                start=(i == 0),
                                stop=(i == niter - 1),
                                skip_group_check=True,
                            )
                        rec = stat.tile([128, 1], DTF, tag="rec")
                        nc.vector.reciprocal(rec[:], ps_d[:])
                        ao = stage.tile([128, 128], DTB, tag="ao")
                        nc.scalar.activation(
                            ao[:], ps_o[:], AF.Copy, scale=rec[:]
                        )
                        nc.sync.dma_start_transpose(
                            attn[
                                :,
                                h * TPC
                                + chunk * (TPC // 2)
                                + b * 128 : h * TPC
                                + chunk * (TPC // 2)
                                + b * 128
                                + 128,
                            ],
                            ao[:],
                        )

        # ================= phase 4: proj + residual + LN2 =================
        att_ps_cm.__exit__(None, None, None)
        aslab_cm.__exit__(None, None, None)
        pq_cm.__exit__(None, None, None)
        mlp_ps_cm = tc.tile_pool(name="mlpps", bufs=4, space="PSUM")
        mlp_ps = mlp_ps_cm.__enter__()
        pr2_cm, pr2 = open_pool("pr2", "left")
        r2b = pr2.tile([128, KT * TPC], DTB)
        for o in range(OT):
            ws = wpool.tile([128, KT * 128], DTB, tag="wslab")
            load_slab(ws, projT, o * 128)
            for c in range(NCOL):
                cs = c * CW
                ps = mlp_ps.tile([128, CW], DTF, tag="ps")
                for k in range(KT):
                    nc.tensor.matmul(
                        ps[:],
                        lhsT=ws[:, k * 128 : (k + 1) * 128],
                        rhs=attn[:, k * TPC + cs : k * TPC + cs + CW],
                        start=(k == 0),
                        stop=(k == KT - 1),
                    )
                xt = scr.tile([128, CW], DTF, tag="xt")
                nc.sync.dma_start(
                    xt[:], x_f32[o * 128 : (o + 1) * 128, cs : cs + CW]
                )
                r2t = stage.tile([128, CW], DTF, tag="r2t")
                nc.vector.scalar_tensor_tensor(
                    r2t[:],
                    ps[:],
                    pb[:, o : o + 1],
                    xt[:],
                    op0=mybir.AluOpType.add,
                    op1=mybir.AluOpType.add,
                )
                nc.sync.dma_start(
                    r2_dram[o * 128 : (o + 1) * 128, cs : cs + CW], r2t[:]
                )
                nc.scalar.copy(r2b[:, o * TPC + cs : o * TPC + cs + CW], r2t[:])
        pat_cm.__exit__(None, None, None)
        ph2_cm, ph2 = open_pool("ph2", "right")
        h2 = ph2.tile([128, KT * TPC], DTB)
        with tc.tile_pool(name="lnS2", bufs=1, space="PSUM") as pS2, tc.tile_pool(
            name="lnB2", bufs=1, space="PSUM"
        ) as pB2:
            ln_fm(r2b, h2, pS2, pB2)
        pr2_cm.__exit__(None, None, None)

        # ================= phase 5: MLP =================
        for c in range(NCOL):
            cs = c * CW
            if c == 0:
                pg_cm, pg = open_pool("pg", "left")
            g = pg.tile([128, FT * CW], DTB, tag="gelu")
            for o in range(FT):
                ws = wpool.tile([128, KT * 128], DTB, tag="wslab")
                load_slab(ws, fc1T, o * 128)
                ps = mlp_ps.tile([128, CW], DTF, tag="ps")
                for k in range(KT):
                    nc.tensor.matmul(
                        ps[:],
                        lhsT=ws[:, k * 128 : (k + 1) * 128],
                        rhs=h2[:, k * TPC + cs : k * TPC + cs + CW],
                        start=(k == 0),
                        stop=(k == KT - 1),
                    )
                nc.scalar.activation(
                    g[:, o * CW : (o + 1) * CW],
                    ps[:],
                    AF.Gelu,
                    bias=f1b[:, o : o + 1],
                )
            for o in range(OT):
                ws2 = wpool.tile([128, max(KT * 512, FT * 128)], DTB, tag="bigslab", bufs=2, name="w2")[ : , : FT * 128]
                load_slab(ws2, fc2T, o * 128)
                ps = mlp_ps.tile([128, CW], DTF, tag="ps")
                for k in range(FT):
                    nc.tensor.matmul(
                        ps[:],
                        lhsT=ws2[:, k * 128 : (k + 1) * 128],
                        rhs=g[:, k * CW : (k + 1) * CW],
                        start=(k == 0),
                        stop=(k == FT - 1),
                    )
                rt = scr.tile([128, CW], DTF, tag="rt")
                nc.sync.dma_start(
                    rt[:], r2_dram[o * 128 : (o + 1) * 128, cs : cs + CW]
                )
                yt = stage.tile([128, CW], DTF, tag="yt")
                nc.vector.scalar_tensor_tensor(
                    yt[:],
                    ps[:],
                    f2b[:, o : o + 1],
                    rt[:],
                    op0=mybir.AluOpType.add,
                    op1=mybir.AluOpType.add,
                )
                nc.sync.dma_start(y_out[o * 128 : (o + 1) * 128, cs : cs + CW], yt[:])
        pg_cm.__exit__(None, None, None)
        ph2_cm.__exit__(None, None, None)
        mlp_ps_cm.__exit__(None, None, None)

    nc.compile()
    return nc


# ===================== host side =====================


def _prep(cfg, inputs):
    S, B, H, NH, FF = cfg["S"], cfg["B"], cfg["H"], cfg["NH"], cfg["FF"]
    HD = 128
    NBLK = 2 * NCORE
    TPC = 2 * B * 128
    PAD0, PAD1 = NCORE - 1, NBLK - 1
    f32 = np.float32

    x = np.asarray(inputs["x"], f32)
    ln1_g = np.asarray(inputs["ln1_g"], f32)
    ln1_b = np.asarray(inputs["ln1_b"], f32)
    qkv_w = np.asarray(inputs["qkv_w"], f32)
    qkv_b = np.asarray(inputs["qkv_b"], f32)
    proj_w = np.asarray(inputs["proj_w"], f32)
    proj_b = np.asarray(inputs["proj_b"], f32)
    ln2_g = np.asarray(inputs["ln2_g"], f32)
    ln2_b = np.asarray(inputs["ln2_b"], f32)
    fc1_w = np.asarray(inputs["fc1_w"], f32)
    fc1_b = np.asarray(inputs["fc1_b"], f32)
    fc2_w = np.asarray(inputs["fc2_w"], f32)
    fc2_b = np.asarray(inputs["fc2_b"], f32)

    qkv_w_eff = qkv_w * ln1_g[None, :]
    qkv_b_eff = qkv_b + qkv_w @ ln1_b
    sc = 1.0 / np.sqrt(HD)
    wq, bq = qkv_w_eff[0:H] * sc, qkv_b_eff[0:H] * sc
    wk, bk_ = qkv_w_eff[H : 2 * H], qkv_b_eff[H : 2 * H]
    wv, bv = qkv_w_eff[2 * H : 3 * H], qkv_b_eff[2 * H : 3 * H]
    fc1_w_eff = fc1_w * ln2_g[None, :]
    fc1_b_eff = fc1_b + fc1_w @ ln2_b

    def colmajor_bias(v):  # [X] -> [128, X/128] (partition = feat within tile)
        return np.ascontiguousarray(v.reshape(-1, 128).T).astype(f32)

    shared = dict(
        wqT=np.ascontiguousarray(wq.T).astype(BF),
        wkT=np.ascontiguousarray(wk.T).astype(BF),
        wvT=np.ascontiguousarray(wv.T).astype(BF),
        projT=np.ascontiguousarray(proj_w.T).astype(BF),
        fc1T=np.ascontiguousarray(fc1_w_eff.T).astype(BF),
        fc2T=np.ascontiguousarray(fc2_w.T).astype(BF),
        qb=colmajor_bias(bq),
        kb=colmajor_bias(bk_),
        vb=bv[None, :].astype(BF),
        pb=colmajor_bias(proj_b),
        f1b=colmajor_bias(fc1_b_eff),
        f2b=colmajor_bias(fc2_b),
        tri=np.where(
            np.arange(128)[:, None] > np.arange(128)[None, :], MASKV, 0.0
        ).astype(f32),
        ones_r=np.ones((1, 512), BF),
        ones_f=np.ones((1, 128), f32),
        ones_c=np.ones((128, 1), BF),
    )

    xr = x.reshape(NBLK, 128, B, H)
    in_maps = []
    for c in range(NCORE):
        parts = []
        for chunk in range(2):
            bb = c if chunk == 0 else NBLK - 1 - c
            parts.append(np.ascontiguousarray(xr[bb].transpose(1, 0, 2)).reshape(
                B * 128, H
            ))
        xc = np.concatenate(parts, axis=0)  # [TPC, H]
        xc_fm = np.ascontiguousarray(xc.T)  # [H, TPC]
        bpad = np.zeros((128, PAD0 + PAD1 + 1), f32)
        bpad[:, PAD0 + PAD1] = -EXP_C
        a0, a1 = c, NBLK - 1 - c
        for i in range(PAD0):
            bpad[:, i] = -EXP_C if i < a0 else MASKV
        for i in range(PAD1):
            bpad[:, PAD0 + i] = -EXP_C if i < a1 else MASKV
        m = dict(shared)
        m["x_bf"] = xc_fm.astype(BF)
        m["x_f32"] = xc_fm
        m["bpad"] = bpad
        in_maps.append(m)
    return in_maps


def _assemble(cfg, results):
    S, B, H = cfg["S"], cfg["B"], cfg["H"]
    NBLK = 2 * NCORE
    out = np.zeros((S, B, H), np.float32)
    for c in range(NCORE):
        yc = results[c]["y"]  # [H, TPC]
        yt = yc.T.reshape(2, B, 128, H)  # [chunk, b, s, H]
        for chunk in range(2):
            bb = c if chunk == 0 else NBLK - 1 - c
            out[bb * 128 : (bb + 1) * 128] = yt[chunk].transpose(1, 0, 2)
    return out


_NC_CACHE = {}


def run_cfg(cfg, inputs, trace=False):
    _shim_ntff_hook()
    key = tuple(sorted(cfg.items()))
    if key not in _NC_CACHE:
        _NC_CACHE[key] = build(cfg)
    nc = _NC_CACHE[key]
    in_maps = _prep(cfg, inputs)
    res = bass_utils.run_bass_kernel_spmd(
        nc, in_maps, core_ids=list(range(NCORE)), trace=trace
    )
    _LAST["exec_time_ns"] = res.exec_time_ns
    _LAST["res"] = res
    return _assemble(cfg, res.results)


def kernel(**inputs):
    return run_cfg(FULL, inputs, trace=False)

